# revision 20
# baseline (speedup 1.0000x reference)
"""DeltaRule memory scan kernel for Trainium2, 8 NeuronCores, data-parallel over batch.

Reference semantics (per batch element, H=512, L=2048):
    M_0 = 0  [H,H]
    for t in 0..L-2:   k = hidden[t]
        d = k.k + eps; delta = k - (M k)/d; M += outer(delta, k)
    out = (M @ hidden[L-1]) @ W.T + b

Implementation: chunked delta rule (UT transform), chunk C=128, fused into
macro-chunks of 256 tokens via block inversion:
    T = [[T1, 0], [-T2 A21 T1, T2]],  A21 = diag(r2) K2 K1^T
Per chunk with keys K [C,H], r = 1/(rowsum(K^2)+eps):
    A  = strict_tril(diag(r) K K^T)            [C,C]
    T^T ~= Neumann-by-squaring, NLEV=1 (exact through A^3; validated in
           numpy simulation + CoreSim: max rel err ~0.013-0.016 < 2e-2 gate.
           NLEV=2 costs ~34us more and is only needed below ~1e-2)
    U1 = K1 - diag(r1) (K1 M^T);          Dl1 = T1 U1
    U2 = K2 - diag(r2) (K2 M^T + S Dl1),  S = K2 K1^T;  Dl2 = T2 U2
    M^T += K1^T Dl1 + K2^T Dl2            (ONE state accumulate per 256 tokens)
Dtype strategy: the M^T state lives in float32r (fp32 with ~12-bit mantissa,
1 cycle/row on the PE when the moving free dim >= 256) and is accumulated
in-place by DVE tensor_add writing f32r — no bf16 shadow copy of the state.
K^T also gets a f32r copy (state_u stationary). Everything else (A chain,
Dl, U, S) is bf16. PSUM->SBUF copies are spread across Scalar(ACT),
GpSimd(Pool) and DVE so the PE (~11.5us per 128-chunk) is the bottleneck.
Build of macro m+1 (DMA, A, S, Neumann chains) interleaves into the state
phase of macro m. 4 batch elements per core.
"""
import sys
import numpy as np
from contextlib import ExitStack

sys.path.insert(0, "/opt/trn_rl_repo")

import concourse.bass as bass
import concourse.mybir as mybir
from concourse import tile
from concourse.bass_utils import run_bass_kernel_spmd
from concourse.masks import make_identity, make_lower_triangular

B, L, H = 32, 2048, 512
NCORES = 8
BPC = B // NCORES          # 4 batch elements per core
C = 128                    # chunk size
T = L - 1                  # 2047 scan steps
NCHUNK = (T + C - 1) // C  # 16 (last chunk has 127 valid rows)
NMACRO = NCHUNK // 2       # 8 macro-chunks of 256 tokens
NLEV = 1                   # Neumann levels -> exact through A^3 (validated: max rel ~0.013)
EPS = 1e-6
HB = H // 128              # 4 h-blocks

f32 = mybir.dt.float32
f32r = mybir.dt.float32r
bf16 = mybir.dt.bfloat16
MULT = mybir.AluOpType.mult
ADD = mybir.AluOpType.add

_cached = {}


def _build_program(legalize=True):
    nc = bass.Bass(target_bir_lowering=False, debug=False)

    hidden_d = nc.declare_dram_parameter("hidden", [BPC, L, H], f32, isOutput=False)
    w_d = nc.declare_dram_parameter("W", [H, H], f32, isOutput=False)
    b_d = nc.declare_dram_parameter("bvec", [H], f32, isOutput=False)
    z_d = nc.declare_dram_parameter("zrow", [1, H], f32, isOutput=False)
    out_d = nc.declare_dram_parameter("out", [BPC, H], f32, isOutput=True)

    with tile.TileContext(nc) as tc, ExitStack() as ctx:
        consts = ctx.enter_context(tc.tile_pool(name="consts", bufs=1))
        wbuild = ctx.enter_context(tc.tile_pool(name="wbuild", bufs=2))
        mtpool = ctx.enter_context(tc.tile_pool(name="mt", bufs=1))
        kpool = ctx.enter_context(tc.tile_pool(name="k", bufs=16))
        chain = ctx.enter_context(tc.tile_pool(name="chain", bufs=6))
        gpool = ctx.enter_context(tc.tile_pool(name="g", bufs=6))
        dlpool = ctx.enter_context(tc.tile_pool(name="dl", bufs=10))
        upool = ctx.enter_context(tc.tile_pool(name="u", bufs=6))
        stpool = ctx.enter_context(tc.tile_pool(name="st", bufs=3))
        small = ctx.enter_context(tc.tile_pool(name="small", bufs=16))
        pslo = ctx.enter_context(tc.tile_pool(name="pslo", bufs=2, space="PSUM"))
        pshi = ctx.enter_context(tc.tile_pool(name="pshi", bufs=6, space="PSUM"))

        # ---- constants ----
        ident_f = consts.tile([128, 128], f32, tag="identf")
        make_identity(nc, ident_f[:])
        ident_b = consts.tile([128, 128], bf16, tag="identb")
        make_identity(nc, ident_b[:])
        identp = consts.tile([128, H], bf16, tag="identp")
        for bi in range(BPC):
            nc.vector.tensor_copy(identp[:, bi * 128:(bi + 1) * 128], ident_b[:])
        smask = consts.tile([128, 128], f32, tag="smask")
        make_lower_triangular(nc, smask[:], val=1.0, diag=False)

        # W^T in f32r: WT[ib][i', o] = W[o, ib*128+i']
        wt = [consts.tile([128, H], f32r, tag=f"wt{ib}", name=f"wt{ib}") for ib in range(HB)]
        for op in range(HB):
            wsb = wbuild.tile([128, H], f32, tag="wsb")
            nc.sync.dma_start(wsb[:], w_d[op * 128:(op + 1) * 128, :])
            for ib in range(HB):
                tps = pslo.tile([128, 128], f32, tag="sm")
                nc.tensor.transpose(tps[:], wsb[:, ib * 128:(ib + 1) * 128], ident_f[:])
                nc.vector.tensor_copy(wt[ib][:, op * 128:(op + 1) * 128], tps[:])

        bias_row = consts.tile([1, H], f32, tag="biasrow")
        nc.sync.dma_start(bias_row[:], b_d[None, :])

        # q[b] as [128, HB] f32r column tile (q_t[p, jb] = q[jb*128+p])
        qs = []
        for bi in range(BPC):
            v4 = wbuild.tile([HB, 128], f32, tag="v4")
            nc.sync.dma_start(v4[:], hidden_d[bi, L - 1, :].rearrange("(f p) -> f p", p=128))
            tps = pslo.tile([128, HB], f32, tag="sm")
            nc.tensor.transpose(tps[:], v4[:], ident_f[:HB, :HB])
            q_t = consts.tile([128, HB], f32r, tag=f"q{bi}", name=f"q{bi}")
            nc.scalar.copy(q_t[:], tps[:])
            qs.append(q_t)

        # ---- state: M^T per (b, jb) in f32r, accumulated in-place by DVE ----
        mts = [[mtpool.tile([128, H], f32r, tag=f"mt{bi}_{jb}", name=f"mt{bi}_{jb}")
                for jb in range(HB)] for bi in range(BPC)]

        G = {}  # per-chunk live tiles

        def prep(c):
            t0 = c * C
            nrows = min(C, T - t0)
            st = {"k": [], "kb": [], "ktf": [], "ktb": [], "r": [], "nr": []}
            for bi in range(BPC):
                k_t = kpool.tile([128, H], f32, tag="K", name=f"k{c}_{bi}")
                # always DMA a full 128-row tile (a partial-row DMA degrades to
                # per-row descriptors); overwrite the out-of-range rows with a
                # zero row from DRAM (engines cannot address partition 127)
                nc.sync.dma_start(k_t[:], hidden_d[bi, t0:t0 + C, :])
                for z in range(nrows, C):
                    nc.sync.dma_start(k_t[z:z + 1, :], z_d[0:1, :])
                st["k"].append(k_t)
                kb = kpool.tile([128, H], bf16, tag="Kb", name=f"kb{c}_{bi}")
                # ramp: GPS cast is 1.9us/tile and serializes the pipeline
                # fill; DVE is idle during the first two macros
                if c < 4:
                    nc.vector.tensor_copy(kb[:], k_t[:])
                else:
                    nc.gpsimd.tensor_copy(kb[:], k_t[:])
                st["kb"].append(kb)
                scr = small.tile([128, H], bf16, tag="scr", bufs=2)
                d_t = small.tile([128, 1], f32, tag="d", bufs=4)
                nc.scalar.activation(scr[:], kb[:], mybir.ActivationFunctionType.Square,
                                     accum_out=d_t[:])
                r_t = small.tile([128, 1], f32, tag="r")
                nc.vector.tensor_scalar_add(d_t[:], d_t[:], EPS)
                nc.vector.reciprocal(r_t[:], d_t[:])
                nr_t = small.tile([128, 1], f32, tag="nr")
                nc.vector.tensor_scalar_mul(nr_t[:], r_t[:], -1.0)
                st["r"].append(r_t); st["nr"].append(nr_t)
                ktps = pshi.tile([128, H], bf16, tag="big")
                for hb in range(HB):
                    nc.tensor.transpose(ktps[:, hb * 128:(hb + 1) * 128],
                                        kb[:, hb * 128:(hb + 1) * 128], ident_b[:])
                ktf = kpool.tile([128, H], f32r, tag="ktf", name=f"ktf{c}_{bi}")
                nc.scalar.copy(ktf[:], ktps[:])
                st["ktf"].append(ktf)
                ktb = kpool.tile([128, H], bf16, tag="ktb", name=f"ktb{c}_{bi}")
                nc.scalar.copy(ktb[:], ktf[:])
                st["ktb"].append(ktb)
            G[c] = st

        def aform(c):
            st = G[c]
            a_ps = pshi.tile([128, H], f32, tag="big")
            for bi in range(BPC):
                sl = slice(bi * 128, (bi + 1) * 128)
                for hb in range(HB):
                    nc.tensor.matmul(a_ps[:, sl], st["ktb"][bi][:, hb * 128:(hb + 1) * 128],
                                     st["ktb"][bi][:, hb * 128:(hb + 1) * 128],
                                     start=(hb == 0), stop=(hb == HB - 1))
            a_all = chain.tile([128, H], bf16, tag="ak")
            for bi in range(BPC):
                sl = slice(bi * 128, (bi + 1) * 128)
                nc.vector.scalar_tensor_tensor(a_all[:, sl], a_ps[:, sl], st["r"][bi][:],
                                               smask[:], MULT, MULT)
            at_ps = pshi.tile([128, H], bf16, tag="big")
            for bi in range(BPC):
                sl = slice(bi * 128, (bi + 1) * 128)
                nc.tensor.transpose(at_ps[:, sl], a_all[:, sl], ident_b[:])
            at_all = chain.tile([128, H], bf16, tag="atk")
            nc.scalar.copy(at_all[:], at_ps[:])
            g0 = gpool.tile([128, H], bf16, tag="g")
            nc.vector.tensor_sub(g0[:], identp[:], at_all[:])
            st["ak"], st["atk"], st["g"] = a_all, at_all, g0

        def stb_form(m):
            # S^T = K1 K2^T per bi (chunks 2m, 2m+1), bf16, packed [128, 4*128]
            sa, sb = G[2 * m], G[2 * m + 1]
            s_ps = pshi.tile([128, H], f32, tag="big")
            for bi in range(BPC):
                sl = slice(bi * 128, (bi + 1) * 128)
                for hb in range(HB):
                    nc.tensor.matmul(s_ps[:, sl], sa["ktb"][bi][:, hb * 128:(hb + 1) * 128],
                                     sb["ktb"][bi][:, hb * 128:(hb + 1) * 128],
                                     start=(hb == 0), stop=(hb == HB - 1))
            stb = stpool.tile([128, H], bf16, tag="stb", name=f"stb{m}")
            nc.scalar.copy(stb[:], s_ps[:])
            sa["stb"] = stb

        def bt_form(m):
            # B^T = T1^T S^T per bi, so SU_b can use U1 instead of Dl1:
            # S Dl1 = S (T1 U1) = (S T1) U1, with lhsT = (S T1)^T = T1^T S^T
            sa = G[2 * m]
            gat_ps = pshi.tile([128, H], bf16, tag="big")
            for bi in range(BPC):
                sl = slice(bi * 128, (bi + 1) * 128)
                nc.tensor.transpose(gat_ps[:, sl], sa["g"][:, sl], ident_b[:])
            gat = stpool.tile([128, H], bf16, tag="gat", bufs=2)
            nc.scalar.copy(gat[:], gat_ps[:])
            bt_ps = pshi.tile([128, H], f32, tag="big")
            for bi in range(BPC):
                sl = slice(bi * 128, (bi + 1) * 128)
                nc.tensor.matmul(bt_ps[:, sl], gat[:, sl], sa["stb"][:, sl],
                                 start=True, stop=True)
            btb = stpool.tile([128, H], bf16, tag="btb", name=f"btb{m}")
            nc.scalar.copy(btb[:], bt_ps[:])
            sa["btb"] = btb

        def chain_sq(c, lev):
            # squarings for one level: ak2 = ak@ak, atk2 = (ak@ak)^T
            st = G[c]
            ak, atk = st["ak"], st["atk"]
            sq1 = pshi.tile([128, H], f32, tag="big")
            for bi in range(BPC):
                sl = slice(bi * 128, (bi + 1) * 128)
                nc.tensor.matmul(sq1[:, sl], atk[:, sl], ak[:, sl], start=True, stop=True)
            ak2 = chain.tile([128, H], bf16, tag="ak")
            nc.scalar.copy(ak2[:], sq1[:])
            if lev < NLEV:
                sq2 = pshi.tile([128, H], f32, tag="big")
                for bi in range(BPC):
                    sl = slice(bi * 128, (bi + 1) * 128)
                    nc.tensor.matmul(sq2[:, sl], ak[:, sl], atk[:, sl], start=True, stop=True)
                atk2 = chain.tile([128, H], bf16, tag="atk")
                nc.scalar.copy(atk2[:], sq2[:])
            else:
                atk2 = None
            st["ak2_n"], st["atk2_n"] = ak2, atk2

        def chain_g(c):
            # g update with the freshly squared ak2 (copy latency hidden by the
            # other chunk's squarings emitted in between)
            st = G[c]
            ak2, atk2 = st.pop("ak2_n"), st.pop("atk2_n")
            gps = pshi.tile([128, H], f32, tag="big")
            for bi in range(BPC):
                sl = slice(bi * 128, (bi + 1) * 128)
                nc.tensor.matmul(gps[:, sl], ak2[:, sl], st["g"][:, sl], start=True, stop=True)
            g_nxt = gpool.tile([128, H], bf16, tag="g")
            nc.vector.tensor_add(g_nxt[:], gps[:], st["g"][:])
            st["ak"], st["atk"], st["g"] = ak2, atk2, g_nxt

        def chain_pair(ca, cb, lev):
            chain_sq(ca, lev)
            chain_sq(cb, lev)
            chain_g(ca)
            chain_g(cb)

        def state_u_first(c, m):
            # first chunk of macro: U = K - diag(r) K M^T  (u in bf16)
            st = G[c]
            st["u"] = []
            for bi in range(BPC):
                if m == 0:
                    st["u"].append(st["kb"][bi])  # M = 0 -> U = K
                    continue
                ups = pshi.tile([128, H], f32, tag="big")
                for hb in range(HB):
                    nc.tensor.matmul(ups[:], st["ktf"][bi][:, hb * 128:(hb + 1) * 128],
                                     mts[bi][hb][:], start=(hb == 0), stop=(hb == HB - 1))
                u_sb = upool.tile([128, H], bf16, tag="u")
                nc.vector.scalar_tensor_tensor(u_sb[:], ups[:], st["nr"][bi][:],
                                               st["k"][bi][:], MULT, ADD)
                st["u"].append(u_sb)

        def state_u_second(c, m):
            # second chunk: U = K - diag(r) (K M^T + (S T1) U1)
            st = G[c]
            sa = G[2 * m]
            st["u"] = []
            for bi in range(BPC):
                sl = slice(bi * 128, (bi + 1) * 128)
                ups = pshi.tile([128, H], f32, tag="big")
                if m > 0:
                    for hb in range(HB):
                        nc.tensor.matmul(ups[:], st["ktf"][bi][:, hb * 128:(hb + 1) * 128],
                                         mts[bi][hb][:], start=(hb == 0), stop=False)
                nc.tensor.matmul(ups[:], sa["btb"][:, sl], sa["u"][bi][:],
                                 start=(m == 0), stop=True)
                u_sb = upool.tile([128, H], bf16, tag="u")
                nc.vector.scalar_tensor_tensor(u_sb[:], ups[:], st["nr"][bi][:],
                                               st["k"][bi][:], MULT, ADD)
                st["u"].append(u_sb)

        def state_delta(c, copy_engine):
            st = G[c]
            st["dl"] = []
            for bi in range(BPC):
                sl = slice(bi * 128, (bi + 1) * 128)
                dps = pshi.tile([128, H], f32, tag="big")
                nc.tensor.matmul(dps[:], st["g"][:, sl], st["u"][bi][:], start=True, stop=True)
                dl_sb = dlpool.tile([128, H], bf16, tag="dl")
                copy_engine(dl_sb[:], dps[:])
                st["dl"].append(dl_sb)

        def state_mupd(m, bis):
            # M^T += K1^T Dl1 + K2^T Dl2 (both matmuls accumulate in PSUM)
            sa, sb = G[2 * m], G[2 * m + 1]
            for bi in bis:
                for jb in range(HB):
                    mps = pshi.tile([128, H], f32, tag="big")
                    nc.tensor.matmul(mps[:], sa["kb"][bi][:, jb * 128:(jb + 1) * 128],
                                     sa["dl"][bi][:], start=True, stop=False)
                    nc.tensor.matmul(mps[:], sb["kb"][bi][:, jb * 128:(jb + 1) * 128],
                                     sb["dl"][bi][:], start=False, stop=True)
                    if m == 0:
                        nc.vector.tensor_copy(mts[bi][jb][:], mps[:])
                    else:
                        nc.vector.tensor_add(mts[bi][jb][:], mps[:], mts[bi][jb][:])

        def build_macro(m):
            ca, cb = 2 * m, 2 * m + 1
            return [
                lambda: prep(ca),
                lambda: prep(cb),
                lambda: aform(ca),
                lambda: (aform(cb), stb_form(m)),
                lambda: chain_pair(ca, cb, 1),
            ]

        def state_macro(m):
            ca, cb = 2 * m, 2 * m + 1
            return [
                lambda: state_u_first(ca, m),
                lambda: state_u_second(cb, m),
                lambda: state_delta(ca, nc.vector.tensor_copy),
                lambda: state_delta(cb, nc.scalar.copy),
                lambda: state_mupd(m, [0, 1]),
                lambda: state_mupd(m, [2, 3]),
            ]

        def finale(bi):
            # ctx = M q (row form); out = ctx W^T + b
            cps = pshi.tile([1, H], f32, tag="big")
            for jb in range(HB):
                nc.tensor.matmul(cps[:], qs[bi][:, jb:jb + 1], mts[bi][jb][:],
                                 start=(jb == 0), stop=(jb == HB - 1))
            ctx_row = small.tile([1, H], f32, tag="ctxrow", bufs=2)
            nc.scalar.copy(ctx_row[:], cps[:])
            ctxT = small.tile([128, HB], f32r, tag="ctxT", bufs=2)
            for ib in range(HB):
                tp2 = pslo.tile([128, 1], f32, tag="sm")
                nc.tensor.transpose(tp2[:], ctx_row[:, ib * 128:(ib + 1) * 128], ident_f[:1, :1])
                nc.scalar.copy(ctxT[:, ib:ib + 1], tp2[:])
            ops_ = pshi.tile([1, H], f32, tag="big")
            for ib in range(HB):
                nc.tensor.matmul(ops_[:], ctxT[:, ib:ib + 1], wt[ib][:],
                                 start=(ib == 0), stop=(ib == HB - 1))
            out_row = small.tile([1, H], f32, tag="outrow", bufs=2)
            nc.vector.tensor_add(out_row[:], ops_[:], bias_row[:])
            nc.sync.dma_start(out_d[bi, :][None, :], out_row[:])

        for f in build_macro(0):
            f()
        bt_form(0)
        for m in range(NMACRO):
            build = build_macro(m + 1) if m + 1 < NMACRO else []
            state = state_macro(m)
            last = m == NMACRO - 1
            order = [("s", 0), ("b", 0), ("s", 1), ("b", 1), ("s", 2), ("b", 2),
                     ("s", 3), ("b", 3), ("s", 4), ("b", 4),
                     ("f", 0), ("f", 1),
                     ("s", 5), ("b", 5), ("bt", 0),
                     ("f", 2), ("f", 3)]
            for kind, i in order:
                if kind == "s":
                    state[i]()
                elif kind == "f":
                    if last:
                        finale(i)
                elif kind == "bt":
                    if m + 1 < NMACRO:
                        bt_form(m + 1)
                elif i < len(build):
                    build[i]()
            del G[2 * m]
            del G[2 * m + 1]

    if legalize:
        _legalize_waits(nc)
    return nc


def _legalize_waits(nc, max_waits=1):
    """This toolchain's walrus encodes at most one semaphore wait per
    instruction. Hoist extra waits onto standalone EventSemaphore
    instructions on the same engine queue, immediately before the owner."""
    import json as _json
    m = _json.loads(bytes(nc.to_json_bytes()))
    n_fix = 0
    for fn in m["functions"]:
        for blk in fn["blocks"]:
            out = []
            for ins in blk.get("instructions", []):
                si = ins.get("sync_info") or {}
                waits = si.get("on_wait") or []
                if len(waits) > max_waits and ins.get("opcode") != "EventSemaphore":
                    extra, keep = waits[:-max_waits], waits[-max_waits:]
                    for i, w in enumerate(extra):
                        out.append({
                            "name": f"{ins['name']}-w{i}",
                            "engine": ins["engine"],
                            "opcode": "EventSemaphore",
                            "ins": [], "outs": [],
                            "sync_info": {"on_wait": [w], "on_update": []},
                        })
                    si["on_wait"] = keep
                    ins["sync_info"] = si
                    n_fix += 1
                out.append(ins)
            blk["instructions"] = out
    nc.m = mybir.module_from_json_bytes(_json.dumps(m).encode())
    return n_fix


def kernel(hidden: np.ndarray, W: np.ndarray, b: np.ndarray) -> np.ndarray:
    if "nc" not in _cached:
        _cached["nc"] = _build_program()
    nc = _cached["nc"]

    hidden = np.ascontiguousarray(hidden, dtype=np.float32)
    W = np.ascontiguousarray(W, dtype=np.float32)
    b = np.ascontiguousarray(b, dtype=np.float32)

    in_maps = []
    for ci in range(NCORES):
        in_maps.append({
            "hidden": hidden[ci * BPC:(ci + 1) * BPC],
            "W": W,
            "bvec": b,
            "zrow": np.zeros((1, H), np.float32),
        })
    res = run_bass_kernel_spmd(nc, in_maps, core_ids=list(range(NCORES)))
    _cached["last_results"] = res
    out = np.concatenate([res.results[ci]["out"] for ci in range(NCORES)], axis=0)
    return out.astype(np.float32)


if __name__ == "__main__":
    rng = np.random.default_rng(0)
    h = rng.standard_normal((B, L, H), dtype=np.float32)
    w = rng.standard_normal((H, H), dtype=np.float32) * (1.0 / np.sqrt(H))
    bb = np.zeros((H,), np.float32)
    o = kernel(h, w, bb)
    print(o.shape, o.dtype)



# revision 22
# speedup vs baseline: 1.0783x; 1.0783x over previous
"""DeltaRule memory scan kernel for Trainium2, 8 NeuronCores, data-parallel over batch.

Reference semantics (per batch element, H=512, L=2048):
    M_0 = 0  [H,H]
    for t in 0..L-2:   k = hidden[t]
        d = k.k + eps; delta = k - (M k)/d; M += outer(delta, k)
    out = (M @ hidden[L-1]) @ W.T + b

Implementation: chunked delta rule (UT transform), chunk C=128, fused into
macro-chunks of 256 tokens via block inversion:
    T = [[T1, 0], [-T2 A21 T1, T2]],  A21 = diag(r2) K2 K1^T
Per chunk with keys K [C,H], r = 1/(rowsum(K^2)+eps):
    A  = strict_tril(diag(r) K K^T)            [C,C]
    T^T ~= Neumann-by-squaring, NLEV=1 (exact through A^3; validated in
           numpy simulation + CoreSim: max rel err ~0.013-0.016 < 2e-2 gate.
           NLEV=2 costs ~34us more and is only needed below ~1e-2)
    U1 = K1 - diag(r1) (K1 M^T);          Dl1 = T1 U1
    U2 = K2 - diag(r2) (K2 M^T + S Dl1),  S = K2 K1^T;  Dl2 = T2 U2
    M^T += K1^T Dl1 + K2^T Dl2            (ONE state accumulate per 256 tokens)
Dtype strategy: the M^T state lives in float32r (fp32 with ~12-bit mantissa,
1 cycle/row on the PE when the moving free dim >= 256) and is accumulated
in-place by DVE tensor_add writing f32r — no bf16 shadow copy of the state.
K^T also gets a f32r copy (state_u stationary). Everything else (A chain,
Dl, U, S) is bf16. PSUM->SBUF copies are spread across Scalar(ACT),
GpSimd(Pool) and DVE so the PE (~11.5us per 128-chunk) is the bottleneck.
Build of macro m+1 (DMA, A, S, Neumann chains) interleaves into the state
phase of macro m. 4 batch elements per core.
"""
import sys
import numpy as np
from contextlib import ExitStack

sys.path.insert(0, "/opt/trn_rl_repo")

import concourse.bass as bass
import concourse.mybir as mybir
from concourse import tile
from concourse.bass_utils import run_bass_kernel_spmd
from concourse.masks import make_identity, make_lower_triangular

B, L, H = 32, 2048, 512
NCORES = 8
BPC = B // NCORES          # 4 batch elements per core
C = 128                    # chunk size
T = L - 1                  # 2047 scan steps
NCHUNK = (T + C - 1) // C  # 16 (last chunk has 127 valid rows)
NMACRO = NCHUNK // 2       # 8 macro-chunks of 256 tokens
NLEV = 1                   # Neumann levels -> exact through A^3 (validated: max rel ~0.013)
EPS = 1e-6
HB = H // 128              # 4 h-blocks

f32 = mybir.dt.float32
f32r = mybir.dt.float32r
bf16 = mybir.dt.bfloat16
MULT = mybir.AluOpType.mult
ADD = mybir.AluOpType.add

_cached = {}


def _build_program(legalize=True):
    nc = bass.Bass(target_bir_lowering=False, debug=False)

    hidden_d = nc.declare_dram_parameter("hidden", [BPC, L, H], f32, isOutput=False)
    w_d = nc.declare_dram_parameter("W", [H, H], f32, isOutput=False)
    b_d = nc.declare_dram_parameter("bvec", [H], f32, isOutput=False)
    z_d = nc.declare_dram_parameter("zrow", [1, H], f32, isOutput=False)
    out_d = nc.declare_dram_parameter("out", [BPC, H], f32, isOutput=True)

    with tile.TileContext(nc) as tc, ExitStack() as ctx:
        consts = ctx.enter_context(tc.tile_pool(name="consts", bufs=1))
        wbuild = ctx.enter_context(tc.tile_pool(name="wbuild", bufs=2))
        mtpool = ctx.enter_context(tc.tile_pool(name="mt", bufs=1))
        kpool = ctx.enter_context(tc.tile_pool(name="k", bufs=16))
        chain = ctx.enter_context(tc.tile_pool(name="chain", bufs=6))
        gpool = ctx.enter_context(tc.tile_pool(name="g", bufs=6))
        dlpool = ctx.enter_context(tc.tile_pool(name="dl", bufs=10))
        upool = ctx.enter_context(tc.tile_pool(name="u", bufs=6))
        stpool = ctx.enter_context(tc.tile_pool(name="st", bufs=3))
        small = ctx.enter_context(tc.tile_pool(name="small", bufs=16))
        pslo = ctx.enter_context(tc.tile_pool(name="pslo", bufs=2, space="PSUM"))
        pshi = ctx.enter_context(tc.tile_pool(name="pshi", bufs=6, space="PSUM"))

        # ---- constants ----
        ident_f = consts.tile([128, 128], f32, tag="identf")
        make_identity(nc, ident_f[:])
        ident_b = consts.tile([128, 128], bf16, tag="identb")
        make_identity(nc, ident_b[:])
        identp = consts.tile([128, H], bf16, tag="identp")
        for bi in range(BPC):
            nc.vector.tensor_copy(identp[:, bi * 128:(bi + 1) * 128], ident_b[:])
        smask = consts.tile([128, 128], f32, tag="smask")
        make_lower_triangular(nc, smask[:], val=1.0, diag=False)

        # W^T in f32r: WT[ib][i', o] = W[o, ib*128+i']
        wt = [consts.tile([128, H], f32r, tag=f"wt{ib}", name=f"wt{ib}") for ib in range(HB)]
        for op in range(HB):
            wsb = wbuild.tile([128, H], f32, tag="wsb")
            nc.sync.dma_start(wsb[:], w_d[op * 128:(op + 1) * 128, :])
            for ib in range(HB):
                tps = pslo.tile([128, 128], f32, tag="sm")
                nc.tensor.transpose(tps[:], wsb[:, ib * 128:(ib + 1) * 128], ident_f[:])
                nc.vector.tensor_copy(wt[ib][:, op * 128:(op + 1) * 128], tps[:])

        bias_row = consts.tile([1, H], f32, tag="biasrow")
        nc.sync.dma_start(bias_row[:], b_d[None, :])

        # q[b] as [128, HB] f32r column tile (q_t[p, jb] = q[jb*128+p])
        qs = []
        for bi in range(BPC):
            v4 = wbuild.tile([HB, 128], f32, tag="v4")
            nc.sync.dma_start(v4[:], hidden_d[bi, L - 1, :].rearrange("(f p) -> f p", p=128))
            tps = pslo.tile([128, HB], f32, tag="sm")
            nc.tensor.transpose(tps[:], v4[:], ident_f[:HB, :HB])
            q_t = consts.tile([128, HB], f32r, tag=f"q{bi}", name=f"q{bi}")
            nc.scalar.copy(q_t[:], tps[:])
            qs.append(q_t)

        # ---- state: M^T per (b, jb) in f32r, accumulated in-place by DVE ----
        mts = [[mtpool.tile([128, H], f32r, tag=f"mt{bi}_{jb}", name=f"mt{bi}_{jb}")
                for jb in range(HB)] for bi in range(BPC)]

        G = {}  # per-chunk live tiles

        def prep(c):
            t0 = c * C
            nrows = min(C, T - t0)
            st = {"k": [], "kb": [], "ktf": [], "ktb": [], "r": [], "nr": []}
            for bi in range(BPC):
                k_t = kpool.tile([128, H], f32, tag="K", name=f"k{c}_{bi}")
                # always DMA a full 128-row tile (a partial-row DMA degrades to
                # per-row descriptors); overwrite the out-of-range rows with a
                # zero row from DRAM (engines cannot address partition 127)
                nc.sync.dma_start(k_t[:], hidden_d[bi, t0:t0 + C, :])
                for z in range(nrows, C):
                    nc.sync.dma_start(k_t[z:z + 1, :], z_d[0:1, :])
                st["k"].append(k_t)
                scr = small.tile([128, H], bf16, tag="scr", bufs=2)
                d_t = small.tile([128, 1], f32, tag="d", bufs=4)
                nc.scalar.activation(scr[:], k_t[:], mybir.ActivationFunctionType.Square,
                                     accum_out=d_t[:])
                r_t = small.tile([128, 1], f32, tag="r")
                nc.vector.tensor_scalar_add(d_t[:], d_t[:], EPS)
                nc.vector.reciprocal(r_t[:], d_t[:])
                nr_t = small.tile([128, 1], f32, tag="nr")
                nc.vector.tensor_scalar_mul(nr_t[:], r_t[:], -1.0)
                st["r"].append(r_t); st["nr"].append(nr_t)
                kb = kpool.tile([128, H], bf16, tag="Kb", name=f"kb{c}_{bi}")
                nc.gpsimd.tensor_copy(kb[:], k_t[:])
                st["kb"].append(kb)
                ktps = pshi.tile([128, H], bf16, tag="big")
                for hb in range(HB):
                    nc.tensor.transpose(ktps[:, hb * 128:(hb + 1) * 128],
                                        kb[:, hb * 128:(hb + 1) * 128], ident_b[:])
                ktf = kpool.tile([128, H], f32r, tag="ktf", name=f"ktf{c}_{bi}")
                nc.scalar.copy(ktf[:], ktps[:])
                st["ktf"].append(ktf)
                ktb = kpool.tile([128, H], bf16, tag="ktb", name=f"ktb{c}_{bi}")
                nc.scalar.copy(ktb[:], ktf[:])
                st["ktb"].append(ktb)
            G[c] = st

        def aform(c):
            st = G[c]
            a_ps = pshi.tile([128, H], f32, tag="big")
            for bi in range(BPC):
                sl = slice(bi * 128, (bi + 1) * 128)
                for hb in range(HB):
                    nc.tensor.matmul(a_ps[:, sl], st["ktb"][bi][:, hb * 128:(hb + 1) * 128],
                                     st["ktb"][bi][:, hb * 128:(hb + 1) * 128],
                                     start=(hb == 0), stop=(hb == HB - 1))
            a_all = chain.tile([128, H], bf16, tag="ak")
            for bi in range(BPC):
                sl = slice(bi * 128, (bi + 1) * 128)
                nc.vector.scalar_tensor_tensor(a_all[:, sl], a_ps[:, sl], st["r"][bi][:],
                                               smask[:], MULT, MULT)
            at_ps = pshi.tile([128, H], bf16, tag="big")
            for bi in range(BPC):
                sl = slice(bi * 128, (bi + 1) * 128)
                nc.tensor.transpose(at_ps[:, sl], a_all[:, sl], ident_b[:])
            at_all = chain.tile([128, H], bf16, tag="atk")
            nc.scalar.copy(at_all[:], at_ps[:])
            g0 = gpool.tile([128, H], bf16, tag="g")
            nc.vector.tensor_sub(g0[:], identp[:], at_all[:])
            st["ak"], st["atk"], st["g"] = a_all, at_all, g0

        def stb_form(m):
            # S^T = K1 K2^T per bi (chunks 2m, 2m+1), bf16, packed [128, 4*128]
            sa, sb = G[2 * m], G[2 * m + 1]
            s_ps = pshi.tile([128, H], f32, tag="big")
            for bi in range(BPC):
                sl = slice(bi * 128, (bi + 1) * 128)
                for hb in range(HB):
                    nc.tensor.matmul(s_ps[:, sl], sa["ktb"][bi][:, hb * 128:(hb + 1) * 128],
                                     sb["ktb"][bi][:, hb * 128:(hb + 1) * 128],
                                     start=(hb == 0), stop=(hb == HB - 1))
            stb = stpool.tile([128, H], bf16, tag="stb", name=f"stb{m}")
            nc.scalar.copy(stb[:], s_ps[:])
            sa["stb"] = stb

        def bt_form(m):
            # B^T = T1^T S^T per bi, so SU_b can use U1 instead of Dl1:
            # S Dl1 = S (T1 U1) = (S T1) U1, with lhsT = (S T1)^T = T1^T S^T
            sa = G[2 * m]
            gat_ps = pshi.tile([128, H], bf16, tag="big")
            for bi in range(BPC):
                sl = slice(bi * 128, (bi + 1) * 128)
                nc.tensor.transpose(gat_ps[:, sl], sa["g"][:, sl], ident_b[:])
            gat = stpool.tile([128, H], bf16, tag="gat", bufs=2)
            nc.scalar.copy(gat[:], gat_ps[:])
            bt_ps = pshi.tile([128, H], f32, tag="big")
            for bi in range(BPC):
                sl = slice(bi * 128, (bi + 1) * 128)
                nc.tensor.matmul(bt_ps[:, sl], gat[:, sl], sa["stb"][:, sl],
                                 start=True, stop=True)
            btb = stpool.tile([128, H], bf16, tag="btb", name=f"btb{m}")
            nc.scalar.copy(btb[:], bt_ps[:])
            sa["btb"] = btb

        def chain_sq(c, lev):
            # squarings for one level: ak2 = ak@ak, atk2 = (ak@ak)^T
            st = G[c]
            ak, atk = st["ak"], st["atk"]
            sq1 = pshi.tile([128, H], f32, tag="big")
            for bi in range(BPC):
                sl = slice(bi * 128, (bi + 1) * 128)
                nc.tensor.matmul(sq1[:, sl], atk[:, sl], ak[:, sl], start=True, stop=True)
            ak2 = chain.tile([128, H], bf16, tag="ak")
            nc.scalar.copy(ak2[:], sq1[:])
            if lev < NLEV:
                sq2 = pshi.tile([128, H], f32, tag="big")
                for bi in range(BPC):
                    sl = slice(bi * 128, (bi + 1) * 128)
                    nc.tensor.matmul(sq2[:, sl], ak[:, sl], atk[:, sl], start=True, stop=True)
                atk2 = chain.tile([128, H], bf16, tag="atk")
                nc.scalar.copy(atk2[:], sq2[:])
            else:
                atk2 = None
            st["ak2_n"], st["atk2_n"] = ak2, atk2

        def chain_g(c):
            # g update with the freshly squared ak2 (copy latency hidden by the
            # other chunk's squarings emitted in between)
            st = G[c]
            ak2, atk2 = st.pop("ak2_n"), st.pop("atk2_n")
            gps = pshi.tile([128, H], f32, tag="big")
            for bi in range(BPC):
                sl = slice(bi * 128, (bi + 1) * 128)
                nc.tensor.matmul(gps[:, sl], ak2[:, sl], st["g"][:, sl], start=True, stop=True)
            g_nxt = gpool.tile([128, H], bf16, tag="g")
            nc.vector.tensor_add(g_nxt[:], gps[:], st["g"][:])
            st["ak"], st["atk"], st["g"] = ak2, atk2, g_nxt

        def chain_pair(ca, cb, lev):
            chain_sq(ca, lev)
            chain_sq(cb, lev)
            chain_g(ca)
            chain_g(cb)

        def state_u_first(c, m):
            # first chunk of macro: U = K - diag(r) K M^T  (u in bf16)
            st = G[c]
            st["u"] = []
            for bi in range(BPC):
                if m == 0:
                    st["u"].append(st["kb"][bi])  # M = 0 -> U = K
                    continue
                ups = pshi.tile([128, H], f32, tag="big")
                for hb in range(HB):
                    nc.tensor.matmul(ups[:], st["ktf"][bi][:, hb * 128:(hb + 1) * 128],
                                     mts[bi][hb][:], start=(hb == 0), stop=(hb == HB - 1))
                u_sb = upool.tile([128, H], bf16, tag="u")
                nc.vector.scalar_tensor_tensor(u_sb[:], ups[:], st["nr"][bi][:],
                                               st["k"][bi][:], MULT, ADD)
                st["u"].append(u_sb)

        def state_u_second(c, m):
            # second chunk: U = K - diag(r) (K M^T + (S T1) U1)
            st = G[c]
            sa = G[2 * m]
            st["u"] = []
            for bi in range(BPC):
                sl = slice(bi * 128, (bi + 1) * 128)
                ups = pshi.tile([128, H], f32, tag="big")
                if m > 0:
                    for hb in range(HB):
                        nc.tensor.matmul(ups[:], st["ktf"][bi][:, hb * 128:(hb + 1) * 128],
                                         mts[bi][hb][:], start=(hb == 0), stop=False)
                nc.tensor.matmul(ups[:], sa["btb"][:, sl], sa["u"][bi][:],
                                 start=(m == 0), stop=True)
                u_sb = upool.tile([128, H], bf16, tag="u")
                nc.vector.scalar_tensor_tensor(u_sb[:], ups[:], st["nr"][bi][:],
                                               st["k"][bi][:], MULT, ADD)
                st["u"].append(u_sb)

        def state_delta(c, copy_engine):
            st = G[c]
            st["dl"] = []
            for bi in range(BPC):
                sl = slice(bi * 128, (bi + 1) * 128)
                dps = pshi.tile([128, H], f32, tag="big")
                nc.tensor.matmul(dps[:], st["g"][:, sl], st["u"][bi][:], start=True, stop=True)
                dl_sb = dlpool.tile([128, H], bf16, tag="dl")
                copy_engine(dl_sb[:], dps[:])
                st["dl"].append(dl_sb)

        def state_mupd(m, bis):
            # M^T += K1^T Dl1 + K2^T Dl2 (both matmuls accumulate in PSUM)
            sa, sb = G[2 * m], G[2 * m + 1]
            for bi in bis:
                for jb in range(HB):
                    mps = pshi.tile([128, H], f32, tag="big")
                    nc.tensor.matmul(mps[:], sa["kb"][bi][:, jb * 128:(jb + 1) * 128],
                                     sa["dl"][bi][:], start=True, stop=False)
                    nc.tensor.matmul(mps[:], sb["kb"][bi][:, jb * 128:(jb + 1) * 128],
                                     sb["dl"][bi][:], start=False, stop=True)
                    if m == 0:
                        nc.vector.tensor_copy(mts[bi][jb][:], mps[:])
                    else:
                        nc.vector.tensor_add(mts[bi][jb][:], mps[:], mts[bi][jb][:])

        def build_macro(m):
            ca, cb = 2 * m, 2 * m + 1
            return [
                lambda: prep(ca),
                lambda: prep(cb),
                lambda: aform(ca),
                lambda: (aform(cb), stb_form(m)),
                lambda: chain_pair(ca, cb, 1),
            ]

        def state_macro(m):
            ca, cb = 2 * m, 2 * m + 1
            return [
                lambda: state_u_first(ca, m),
                lambda: state_u_second(cb, m),
                lambda: state_delta(ca, nc.vector.tensor_copy),
                lambda: state_delta(cb, nc.scalar.copy),
                lambda: state_mupd(m, [0, 1]),
                lambda: state_mupd(m, [2, 3]),
            ]

        def finale(bi):
            # ctx = M q (row form); out = ctx W^T + b
            cps = pshi.tile([1, H], f32, tag="big")
            for jb in range(HB):
                nc.tensor.matmul(cps[:], qs[bi][:, jb:jb + 1], mts[bi][jb][:],
                                 start=(jb == 0), stop=(jb == HB - 1))
            ctx_row = small.tile([1, H], f32, tag="ctxrow", bufs=2)
            nc.scalar.copy(ctx_row[:], cps[:])
            ctxT = small.tile([128, HB], f32r, tag="ctxT", bufs=2)
            for ib in range(HB):
                tp2 = pslo.tile([128, 1], f32, tag="sm")
                nc.tensor.transpose(tp2[:], ctx_row[:, ib * 128:(ib + 1) * 128], ident_f[:1, :1])
                nc.scalar.copy(ctxT[:, ib:ib + 1], tp2[:])
            ops_ = pshi.tile([1, H], f32, tag="big")
            for ib in range(HB):
                nc.tensor.matmul(ops_[:], ctxT[:, ib:ib + 1], wt[ib][:],
                                 start=(ib == 0), stop=(ib == HB - 1))
            out_row = small.tile([1, H], f32, tag="outrow", bufs=2)
            nc.vector.tensor_add(out_row[:], ops_[:], bias_row[:])
            nc.sync.dma_start(out_d[bi, :][None, :], out_row[:])

        for f in build_macro(0):
            f()
        bt_form(0)
        for m in range(NMACRO):
            build = build_macro(m + 1) if m + 1 < NMACRO else []
            state = state_macro(m)
            last = m == NMACRO - 1
            order = [("s", 0), ("b", 0), ("s", 1), ("b", 1), ("s", 2), ("b", 2),
                     ("s", 3), ("b", 3), ("s", 4), ("b", 4),
                     ("f", 0), ("f", 1),
                     ("s", 5), ("b", 5), ("bt", 0),
                     ("f", 2), ("f", 3)]
            for kind, i in order:
                if kind == "s":
                    state[i]()
                elif kind == "f":
                    if last:
                        finale(i)
                elif kind == "bt":
                    if m + 1 < NMACRO:
                        bt_form(m + 1)
                elif i < len(build):
                    build[i]()
            del G[2 * m]
            del G[2 * m + 1]

    if legalize:
        _legalize_waits(nc)
    return nc


def _legalize_waits(nc, max_waits=1):
    """This toolchain's walrus encodes at most one semaphore wait per
    instruction. Hoist extra waits onto standalone EventSemaphore
    instructions on the same engine queue, immediately before the owner."""
    import json as _json
    m = _json.loads(bytes(nc.to_json_bytes()))
    n_fix = 0
    for fn in m["functions"]:
        for blk in fn["blocks"]:
            out = []
            for ins in blk.get("instructions", []):
                si = ins.get("sync_info") or {}
                waits = si.get("on_wait") or []
                if len(waits) > max_waits and ins.get("opcode") != "EventSemaphore":
                    extra, keep = waits[:-max_waits], waits[-max_waits:]
                    for i, w in enumerate(extra):
                        out.append({
                            "name": f"{ins['name']}-w{i}",
                            "engine": ins["engine"],
                            "opcode": "EventSemaphore",
                            "ins": [], "outs": [],
                            "sync_info": {"on_wait": [w], "on_update": []},
                        })
                    si["on_wait"] = keep
                    ins["sync_info"] = si
                    n_fix += 1
                out.append(ins)
            blk["instructions"] = out
    nc.m = mybir.module_from_json_bytes(_json.dumps(m).encode())
    return n_fix


def kernel(hidden: np.ndarray, W: np.ndarray, b: np.ndarray) -> np.ndarray:
    if "nc" not in _cached:
        _cached["nc"] = _build_program()
    nc = _cached["nc"]

    hidden = np.ascontiguousarray(hidden, dtype=np.float32)
    W = np.ascontiguousarray(W, dtype=np.float32)
    b = np.ascontiguousarray(b, dtype=np.float32)

    in_maps = []
    for ci in range(NCORES):
        in_maps.append({
            "hidden": hidden[ci * BPC:(ci + 1) * BPC],
            "W": W,
            "bvec": b,
            "zrow": np.zeros((1, H), np.float32),
        })
    res = run_bass_kernel_spmd(nc, in_maps, core_ids=list(range(NCORES)))
    _cached["last_results"] = res
    out = np.concatenate([res.results[ci]["out"] for ci in range(NCORES)], axis=0)
    return out.astype(np.float32)


if __name__ == "__main__":
    rng = np.random.default_rng(0)
    h = rng.standard_normal((B, L, H), dtype=np.float32)
    w = rng.standard_normal((H, H), dtype=np.float32) * (1.0 / np.sqrt(H))
    bb = np.zeros((H,), np.float32)
    o = kernel(h, w, bb)
    print(o.shape, o.dtype)

